# revision 1
# baseline (speedup 1.0000x reference)
"""DGL-GAT subgraph encoder kernel for 8 Trainium2 NeuronCores.

With IN_FEATS=1 the GATConv collapses to per-node scalars:
  feat[n,h,d] = f[n]*W1[h,d];  el[n,h] = f[n]*cl[h];  er[n,h] = f[n]*cr[h]
  w[e,h] = exp(lrelu(f[src]*cl[h] + f[dst]*cr[h]))   (softmax max-shift cancels
  in the num/denom ratio; exponents stay < ~25 so no overflow)
  denom[n,h] = seg_sum_dst(w);  num[n,h] = seg_sum_dst(w * f[src])
  s[n,h] = num/denom;  sbar[h] = mean_n s
  out = (sbar[h]*W1[h,:] + bias_gat) @ fc_W + fc_b     (tiny, done on host)

Sharding: core k owns dst nodes [k*12500, (k+1)*12500) and all edges into
them.  Edges are dst-sorted into window-pure 128-edge columns (32-node
one-hot windows, uniform capacity so all cores share one program).  The
device computes per-edge w and w*fs (DVE/ACT) and the two segment sums via
PE matmuls  V[128e,8]^T x onehot[128e,32] accumulated in [8,512] PSUM
blocks; per-core partial (denom,num) tables return to the host, which does
the 100K-node ratio/mean and the final 256x128 projection.
"""
import numpy as np
import ml_dtypes
import concourse.bass as bass
import concourse.tile as tile
from concourse import bacc, mybir, bass_utils

WIN = 32          # nodes per one-hot window (matmul N)
BLK = 512         # nodes per psum block
P = 128           # edges per column
CHK = 128         # columns per onehot chunk
CCH = 512         # columns per compute/load chunk
NCORES = 8

BF16 = ml_dtypes.bfloat16


def _plan(n_nodes, max_win_cnt):
    nodes_pc = -(-n_nodes // NCORES)
    nwin = -(-nodes_pc // WIN)
    ncw = max(1, -(-int(max_win_cnt) // P))
    C = -(-(nwin * ncw) // CCH) * CCH
    nblk = ((C - 1) // ncw) // (BLK // WIN) + 1
    return dict(nodes_pc=nodes_pc, nwin=nwin, ncw=ncw, C=C, nblk=nblk)


def _host_prep_core(f, src_c, dst_c, lo, pl):
    ncw, C = pl["ncw"], pl["C"]
    o = np.argsort(dst_c, kind="stable")
    s_c, d_c = src_c[o], dst_c[o]
    nloc = d_c - lo
    win = nloc >> 5
    starts = np.searchsorted(win, np.arange(pl["nwin"]))
    rank = np.arange(len(win)) - starts[win]
    cap = ncw * P
    assert rank.max(initial=0) < cap, "window capacity overflow"
    flat = win * cap + rank

    def scatter(vals, fill, dt):
        a = np.full(C * P, fill, dtype=np.float32)
        a[flat] = vals
        return np.ascontiguousarray(a.reshape(C, P).T).astype(dt)

    return dict(fs=scatter(f[s_c], 0.0, np.float32),
                fd=scatter(f[d_c], 0.0, np.float32),
                ids=scatter((nloc & 31).astype(np.float32), -1.0, BF16))


def _build_program(pl):
    C, ncw, nblk = pl["C"], pl["ncw"], pl["nblk"]
    nc = bacc.Bacc("TRN2", target_bir_lowering=False, debug=False,
                   enable_asserts=False, num_devices=NCORES)
    bf = mybir.dt.bfloat16
    f32 = mybir.dt.float32

    fs_d = nc.dram_tensor("fs", [P, C], f32, kind="ExternalInput").ap()
    fd_d = nc.dram_tensor("fd", [P, C], f32, kind="ExternalInput").ap()
    ids_d = nc.dram_tensor("ids", [P, C], bf, kind="ExternalInput").ap()
    prm_d = nc.dram_tensor("prm", [P, 8], f32, kind="ExternalInput").ap()
    acc_d = nc.dram_tensor("acc", [8, nblk * BLK], f32, kind="ExternalOutput").ap()
    wpb = BLK // WIN

    with tile.TileContext(nc) as tc:
        with tc.tile_pool(name="consts", bufs=1) as cpool, \
             tc.tile_pool(name="io", bufs=3) as io, \
             tc.tile_pool(name="work", bufs=2) as work, \
             tc.tile_pool(name="ohp", bufs=3) as ohp, \
             tc.tile_pool(name="flp", bufs=2) as flp, \
             tc.tile_pool(name="psum", bufs=2, space="PSUM") as psum_p:
            def flush(blk, ps):
                st = flp.tile([8, BLK], f32, tag="fl")
                nc.scalar.copy(st[:], ps[:])
                nc.sync.dma_start(acc_d[:, blk * BLK:(blk + 1) * BLK], st[:])

            prm = cpool.tile([P, 8], f32, name="prm_s")
            nc.sync.dma_start(prm[:], prm_d)
            iota = cpool.tile([P, WIN], mybir.dt.int16, name="iota_s")
            nc.gpsimd.iota(iota[:], pattern=[[1, WIN]], base=0, channel_multiplier=0)
            iotab = cpool.tile([P, WIN], bf, name="iotab_s")
            nc.vector.tensor_copy(iotab[:], iota[:])

            psum_t, cur_blk = None, -1
            for cc in range(C // CCH):
                c0 = cc * CCH
                fs = io.tile([P, CCH], f32, tag="fs")
                fd = io.tile([P, CCH], f32, tag="fd")
                ids = io.tile([P, CCH], bf, tag="ids")
                nc.sync.dma_start(fs[:], fs_d[:, c0:c0 + CCH])
                nc.sync.dma_start(fd[:], fd_d[:, c0:c0 + CCH])
                nc.sync.dma_start(ids[:], ids_d[:, c0:c0 + CCH])

                vi = work.tile([P, CCH * 8], bf, tag="vi")
                vi3 = vi[:].rearrange("p (c v) -> p c v", v=8)
                t1 = work.tile([P, CCH], f32, tag="t1")
                z = work.tile([P, CCH], f32, tag="z")
                a = work.tile([P, CCH], f32, tag="a")
                wf = work.tile([P, CCH], f32, tag="wf")
                for h in range(4):
                    nc.vector.tensor_scalar_mul(t1[:], fd[:], prm[:, 4 + h:5 + h])
                    nc.vector.scalar_tensor_tensor(
                        out=z[:], in0=fs[:], scalar=prm[:, h:h + 1], in1=t1[:],
                        op0=mybir.AluOpType.mult, op1=mybir.AluOpType.add)
                    nc.vector.scalar_tensor_tensor(
                        out=a[:], in0=z[:], scalar=0.2, in1=z[:],
                        op0=mybir.AluOpType.mult, op1=mybir.AluOpType.max)
                    nc.scalar.activation(wf[:], a[:],
                                         mybir.ActivationFunctionType.Exp)
                    nc.vector.tensor_copy(vi3[:, :, h], wf[:])
                    nc.vector.tensor_mul(vi3[:, :, 4 + h], wf[:], fs[:])

                for ch in range(CCH // CHK):
                    t0 = c0 + ch * CHK
                    oh = ohp.tile([P, CHK * WIN], bf, tag="oh")
                    nc.vector.tensor_tensor(
                        out=oh[:].rearrange("p (c w) -> p c w", w=WIN),
                        in0=ids[:, ch * CHK:(ch + 1) * CHK].unsqueeze(-1)
                            .to_broadcast([P, CHK, WIN]),
                        in1=iotab[:].unsqueeze(1).to_broadcast([P, CHK, WIN]),
                        op=mybir.AluOpType.is_equal)
                    for tl in range(CHK):
                        t = t0 + tl
                        w = t // ncw
                        b = w // wpb
                        if b != cur_blk:
                            if psum_t is not None:
                                flush(cur_blk, psum_t)
                            psum_t = psum_p.tile([8, BLK], f32, tag="ps")
                            cur_blk = b
                        wl = w % wpb
                        nc.tensor.matmul(
                            out=psum_t[:, wl * WIN:(wl + 1) * WIN],
                            lhsT=vi[:, (t - c0) * 8:(t - c0 + 1) * 8],
                            rhs=oh[:, tl * WIN:(tl + 1) * WIN],
                            start=(t % ncw == 0), stop=(t % ncw == ncw - 1))
            flush(cur_blk, psum_t)
    nc.compile()
    return nc


def kernel(features, W, attn_l, attn_r, bias_gat, fc_W, fc_b, src, dst):
    f = np.asarray(features, dtype=np.float32)[:, 0]
    src = np.asarray(src)
    dst = np.asarray(dst)
    N = f.shape[0]
    H, D = np.asarray(attn_l).shape

    nodes_pc = -(-N // NCORES)
    key = ((dst // nodes_pc).astype(np.int64) << 32) | ((dst % nodes_pc) >> 5)
    maxcnt = np.bincount((np.unique(key, return_inverse=True)[1])).max()
    pl = _plan(N, maxcnt)

    W1 = np.asarray(W, np.float64).reshape(H, D)
    cl = (W1 * np.asarray(attn_l, np.float64)).sum(1)
    cr = (W1 * np.asarray(attn_r, np.float64)).sum(1)
    prm = np.zeros((P, 8), dtype=np.float32)
    prm[:, 0:4] = cl.astype(np.float32)
    prm[:, 4:8] = cr.astype(np.float32)

    order = np.argsort(dst, kind="stable")
    ss, dd = src[order], dst[order]
    bounds = np.searchsorted(dd, np.arange(NCORES + 1) * nodes_pc)
    in_maps = []
    for k in range(NCORES):
        a, b = bounds[k], bounds[k + 1]
        arrs = _host_prep_core(f, ss[a:b], dd[a:b], k * nodes_pc, pl)
        in_maps.append({**arrs, "prm": prm})

    nc = _build_program(pl)
    res = bass_utils.run_bass_kernel_spmd(nc, in_maps,
                                          core_ids=list(range(NCORES)),
                                          trace=False)

    ssum = np.zeros(H, dtype=np.float64)
    for k in range(NCORES):
        npc = min(nodes_pc, N - k * nodes_pc)
        acc = res.results[k]["acc"][:, :npc].astype(np.float64)
        denom, num = acc[0:4], acc[4:8]
        s = np.where(denom > 0, num / np.maximum(denom, 1e-300), 0.0)
        ssum += s.sum(axis=1)
    sbar = ssum / N
    rbar = sbar[:, None] * W1 + np.asarray(bias_gat, np.float64).reshape(H, D)
    out = rbar.reshape(1, H * D) @ np.asarray(fc_W, np.float64) \
        + np.asarray(fc_b, np.float64)
    return out[0].astype(np.float32)


# revision 2
# speedup vs baseline: 1.1246x; 1.1246x over previous
"""DGL-GAT subgraph encoder kernel for 8 Trainium2 NeuronCores.

With IN_FEATS=1 the GATConv collapses to per-node scalars:
  feat[n,h,d] = f[n]*W1[h,d];  el[n,h] = f[n]*cl[h];  er[n,h] = f[n]*cr[h]
  w[e,h] = exp(lrelu(f[src]*cl[h] + f[dst]*cr[h]))   (softmax max-shift cancels
  in the num/denom ratio; exponents stay < ~25 so no overflow)
  denom[n,h] = seg_sum_dst(w);  num[n,h] = seg_sum_dst(w * f[src])
  s[n,h] = num/denom;  sbar[h] = mean_n s
  out = (sbar[h]*W1[h,:] + bias_gat) @ fc_W + fc_b     (tiny, done on host)

Sharding: core k owns dst nodes [k*12500, (k+1)*12500) and all edges into
them.  Edges are dst-sorted into window-pure 128-edge columns (32-node
one-hot windows, uniform capacity so all cores share one program).  The
device computes per-edge w and w*fs (DVE/ACT) and the two segment sums via
PE matmuls  V[128e,8]^T x onehot[128e,32] accumulated in [8,512] PSUM
blocks; per-core partial (denom,num) tables return to the host, which does
the 100K-node ratio/mean and the final 256x128 projection.
"""
import numpy as np
import ml_dtypes
import concourse.bass as bass
import concourse.tile as tile
from concourse import bacc, mybir, bass_utils

WIN = 32          # nodes per one-hot window (matmul N)
BLK = 512         # nodes per psum block
P = 128           # edges per column
CHK = 128         # columns per onehot chunk
CCH = 512         # columns per compute/load chunk
NCORES = 8

BF16 = ml_dtypes.bfloat16


def _plan(n_nodes, max_win_cnt):
    nodes_pc = -(-n_nodes // NCORES)
    nwin = -(-nodes_pc // WIN)
    ncw = max(1, -(-int(max_win_cnt) // P))
    C = -(-(nwin * ncw) // CCH) * CCH
    nblk = ((C - 1) // ncw) // (BLK // WIN) + 1
    return dict(nodes_pc=nodes_pc, nwin=nwin, ncw=ncw, C=C, nblk=nblk)


def _host_prep_core(f, src_c, dst_c, lo, pl):
    ncw, C = pl["ncw"], pl["C"]
    o = np.argsort(dst_c, kind="stable")
    s_c, d_c = src_c[o], dst_c[o]
    nloc = d_c - lo
    win = nloc >> 5
    starts = np.searchsorted(win, np.arange(pl["nwin"]))
    rank = np.arange(len(win)) - starts[win]
    cap = ncw * P
    assert rank.max(initial=0) < cap, "window capacity overflow"
    flat = win * cap + rank

    def scatter(vals, fill, dt):
        a = np.full(C * P, fill, dtype=np.float32)
        a[flat] = vals
        return np.ascontiguousarray(a.reshape(C, P).T).astype(dt)

    return dict(fs=scatter(f[s_c], 0.0, np.float32),
                fd=scatter(f[d_c], 0.0, np.float32),
                ids=scatter((nloc & 31).astype(np.float32), -1.0, BF16))


def _build_program(pl):
    C, ncw, nblk = pl["C"], pl["ncw"], pl["nblk"]
    nc = bacc.Bacc("TRN2", target_bir_lowering=False, debug=False,
                   enable_asserts=False, num_devices=NCORES)
    bf = mybir.dt.bfloat16
    f32 = mybir.dt.float32

    fs_d = nc.dram_tensor("fs", [P, C], f32, kind="ExternalInput").ap()
    fd_d = nc.dram_tensor("fd", [P, C], f32, kind="ExternalInput").ap()
    ids_d = nc.dram_tensor("ids", [P, C], bf, kind="ExternalInput").ap()
    prm_d = nc.dram_tensor("prm", [P, 8], f32, kind="ExternalInput").ap()
    acc_d = nc.dram_tensor("acc", [8, nblk * BLK], f32, kind="ExternalOutput").ap()
    wpb = BLK // WIN

    with tile.TileContext(nc) as tc:
        with tc.tile_pool(name="consts", bufs=1) as cpool, \
             tc.tile_pool(name="io", bufs=3) as io, \
             tc.tile_pool(name="work", bufs=2) as work, \
             tc.tile_pool(name="ohp", bufs=3) as ohp, \
             tc.tile_pool(name="flp", bufs=2) as flp, \
             tc.tile_pool(name="psum", bufs=2, space="PSUM") as psum_p:
            def flush(blk, ps):
                st = flp.tile([8, BLK], f32, tag="fl")
                nc.scalar.copy(st[:], ps[:])
                nc.sync.dma_start(acc_d[:, blk * BLK:(blk + 1) * BLK], st[:])

            prm = cpool.tile([P, 8], f32, name="prm_s")
            nc.sync.dma_start(prm[:], prm_d)
            iota = cpool.tile([P, WIN], mybir.dt.int16, name="iota_s")
            nc.gpsimd.iota(iota[:], pattern=[[1, WIN]], base=0, channel_multiplier=0)
            iotab = cpool.tile([P, WIN], bf, name="iotab_s")
            nc.vector.tensor_copy(iotab[:], iota[:])

            psum_t, cur_blk = None, -1
            for cc in range(C // CCH):
                c0 = cc * CCH
                fs = io.tile([P, CCH], f32, tag="fs")
                fd = io.tile([P, CCH], f32, tag="fd")
                ids = io.tile([P, CCH], bf, tag="ids")
                nc.sync.dma_start(fs[:], fs_d[:, c0:c0 + CCH])
                nc.sync.dma_start(fd[:], fd_d[:, c0:c0 + CCH])
                nc.sync.dma_start(ids[:], ids_d[:, c0:c0 + CCH])

                vi = work.tile([P, CCH * 8], bf, tag="vi")
                vi3 = vi[:].rearrange("p (c v) -> p c v", v=8)
                t1 = work.tile([P, CCH], f32, tag="t1")
                z = work.tile([P, CCH], f32, tag="z")
                a = work.tile([P, CCH], f32, tag="a")
                for h in range(4):
                    nc.scalar.mul(t1[:], fd[:], prm[:, 4 + h:5 + h])
                    nc.vector.scalar_tensor_tensor(
                        out=z[:], in0=fs[:], scalar=prm[:, h:h + 1], in1=t1[:],
                        op0=mybir.AluOpType.mult, op1=mybir.AluOpType.add)
                    nc.vector.scalar_tensor_tensor(
                        out=a[:], in0=z[:], scalar=0.2, in1=z[:],
                        op0=mybir.AluOpType.mult, op1=mybir.AluOpType.max)
                    nc.scalar.activation(vi3[:, :, h], a[:],
                                         mybir.ActivationFunctionType.Exp)
                    nc.vector.tensor_mul(vi3[:, :, 4 + h], vi3[:, :, h], fs[:])

                for ch in range(CCH // CHK):
                    t0 = c0 + ch * CHK
                    oh = ohp.tile([P, CHK * WIN], bf, tag="oh")
                    nc.vector.tensor_tensor(
                        out=oh[:].rearrange("p (c w) -> p c w", w=WIN),
                        in0=ids[:, ch * CHK:(ch + 1) * CHK].unsqueeze(-1)
                            .to_broadcast([P, CHK, WIN]),
                        in1=iotab[:].unsqueeze(1).to_broadcast([P, CHK, WIN]),
                        op=mybir.AluOpType.is_equal)
                    for tl in range(CHK):
                        t = t0 + tl
                        w = t // ncw
                        b = w // wpb
                        if b != cur_blk:
                            if psum_t is not None:
                                flush(cur_blk, psum_t)
                            psum_t = psum_p.tile([8, BLK], f32, tag="ps")
                            cur_blk = b
                        wl = w % wpb
                        nc.tensor.matmul(
                            out=psum_t[:, wl * WIN:(wl + 1) * WIN],
                            lhsT=vi[:, (t - c0) * 8:(t - c0 + 1) * 8],
                            rhs=oh[:, tl * WIN:(tl + 1) * WIN],
                            start=(t % ncw == 0), stop=(t % ncw == ncw - 1))
            flush(cur_blk, psum_t)
    nc.compile()
    return nc


def kernel(features, W, attn_l, attn_r, bias_gat, fc_W, fc_b, src, dst):
    f = np.asarray(features, dtype=np.float32)[:, 0]
    src = np.asarray(src)
    dst = np.asarray(dst)
    N = f.shape[0]
    H, D = np.asarray(attn_l).shape

    nodes_pc = -(-N // NCORES)
    key = ((dst // nodes_pc).astype(np.int64) << 32) | ((dst % nodes_pc) >> 5)
    maxcnt = np.bincount((np.unique(key, return_inverse=True)[1])).max()
    pl = _plan(N, maxcnt)

    W1 = np.asarray(W, np.float64).reshape(H, D)
    cl = (W1 * np.asarray(attn_l, np.float64)).sum(1)
    cr = (W1 * np.asarray(attn_r, np.float64)).sum(1)
    prm = np.zeros((P, 8), dtype=np.float32)
    prm[:, 0:4] = cl.astype(np.float32)
    prm[:, 4:8] = cr.astype(np.float32)

    order = np.argsort(dst, kind="stable")
    ss, dd = src[order], dst[order]
    bounds = np.searchsorted(dd, np.arange(NCORES + 1) * nodes_pc)
    in_maps = []
    for k in range(NCORES):
        a, b = bounds[k], bounds[k + 1]
        arrs = _host_prep_core(f, ss[a:b], dd[a:b], k * nodes_pc, pl)
        in_maps.append({**arrs, "prm": prm})

    nc = _build_program(pl)
    res = bass_utils.run_bass_kernel_spmd(nc, in_maps,
                                          core_ids=list(range(NCORES)),
                                          trace=False)

    ssum = np.zeros(H, dtype=np.float64)
    for k in range(NCORES):
        npc = min(nodes_pc, N - k * nodes_pc)
        acc = res.results[k]["acc"][:, :npc].astype(np.float64)
        denom, num = acc[0:4], acc[4:8]
        s = np.where(denom > 0, num / np.maximum(denom, 1e-300), 0.0)
        ssum += s.sum(axis=1)
    sbar = ssum / N
    rbar = sbar[:, None] * W1 + np.asarray(bias_gat, np.float64).reshape(H, D)
    out = rbar.reshape(1, H * D) @ np.asarray(fc_W, np.float64) \
        + np.asarray(fc_b, np.float64)
    return out[0].astype(np.float32)


# revision 4
# speedup vs baseline: 1.1868x; 1.0553x over previous
"""DGL-GAT subgraph encoder kernel for 8 Trainium2 NeuronCores.

With IN_FEATS=1 the GATConv collapses to per-node scalars:
  feat[n,h,d] = f[n]*W1[h,d];  el[n,h] = f[n]*cl[h];  er[n,h] = f[n]*cr[h]
  w[e,h] = exp(lrelu(f[src]*cl[h] + f[dst]*cr[h]))   (softmax max-shift cancels
  in the num/denom ratio; exponents stay < ~25 so no overflow)
  denom[n,h] = seg_sum_dst(w);  num[n,h] = seg_sum_dst(w * f[src])
  s[n,h] = num/denom;  sbar[h] = mean_n s
  out = (sbar[h]*W1[h,:] + bias_gat) @ fc_W + fc_b     (tiny, done on host)

Sharding: core k owns dst nodes [k*12500, (k+1)*12500) and all edges into
them.  Edges are dst-sorted into window-pure 128-edge columns (32-node
one-hot windows, uniform capacity so all cores share one program).  The
device computes per-edge w and w*fs (DVE/ACT) and the two segment sums via
PE matmuls  V[128e,8]^T x onehot[128e,32] accumulated in [8,512] PSUM
blocks; per-core partial (denom,num) tables return to the host, which does
the 100K-node ratio/mean and the final 256x128 projection.
"""
import numpy as np
import ml_dtypes
import concourse.bass as bass
import concourse.tile as tile
from concourse import bacc, mybir, bass_utils

WIN = 32          # nodes per one-hot window (matmul N)
BLK = 512         # nodes per psum block
P = 128           # edges per column
CHK = 128         # columns per onehot chunk
CCH = 512         # columns per compute/load chunk
NCORES = 8

BF16 = ml_dtypes.bfloat16


def _plan(n_nodes, max_win_cnt):
    nodes_pc = -(-n_nodes // NCORES)
    nwin = -(-nodes_pc // WIN)
    ncw = max(1, -(-int(max_win_cnt) // P))
    C = -(-(nwin * ncw) // CCH) * CCH
    nblk = ((C - 1) // ncw) // (BLK // WIN) + 1
    return dict(nodes_pc=nodes_pc, nwin=nwin, ncw=ncw, C=C, nblk=nblk)


def _host_prep_core(f, src_c, dst_c, lo, pl):
    ncw, C = pl["ncw"], pl["C"]
    o = np.argsort(dst_c, kind="stable")
    s_c, d_c = src_c[o], dst_c[o]
    nloc = d_c - lo
    win = nloc >> 5
    starts = np.searchsorted(win, np.arange(pl["nwin"]))
    rank = np.arange(len(win)) - starts[win]
    cap = ncw * P
    assert rank.max(initial=0) < cap, "window capacity overflow"
    flat = win * cap + rank

    def scatter(vals, fill, dt):
        a = np.full(C * P, fill, dtype=np.float32)
        a[flat] = vals
        return np.ascontiguousarray(a.reshape(C, P).T).astype(dt)

    return dict(fs=scatter(f[s_c], 0.0, np.float32),
                fd=scatter(f[d_c], 0.0, np.float32),
                ids=scatter((nloc & 31).astype(np.float32), -1.0, BF16))


def _build_program(pl):
    C, ncw, nblk = pl["C"], pl["ncw"], pl["nblk"]
    nc = bacc.Bacc("TRN2", target_bir_lowering=False, debug=False,
                   enable_asserts=False, num_devices=NCORES)
    bf = mybir.dt.bfloat16
    f32 = mybir.dt.float32

    fs_d = nc.dram_tensor("fs", [P, C], f32, kind="ExternalInput").ap()
    fd_d = nc.dram_tensor("fd", [P, C], f32, kind="ExternalInput").ap()
    ids_d = nc.dram_tensor("ids", [P, C], bf, kind="ExternalInput").ap()
    prm_d = nc.dram_tensor("prm", [P, 8], f32, kind="ExternalInput").ap()
    nsup = -(-nblk // 3)
    acc_d = nc.dram_tensor("acc", [P, nsup * BLK], f32, kind="ExternalOutput").ap()
    wpb = BLK // WIN

    with tile.TileContext(nc) as tc:
        with tc.tile_pool(name="consts", bufs=1) as cpool, \
             tc.tile_pool(name="io", bufs=3) as io, \
             tc.tile_pool(name="work", bufs=2) as work, \
             tc.tile_pool(name="ohp", bufs=3) as ohp, \
             tc.tile_pool(name="flp", bufs=2) as flp, \
             tc.tile_pool(name="psum", bufs=2, space="PSUM") as psum_p:
            def flush(sup, ps):
                st = flp.tile([P, BLK], f32, tag="fl")
                nc.vector.tensor_copy(st[:], ps[:])
                nc.sync.dma_start(acc_d[:, sup * BLK:(sup + 1) * BLK], st[:])

            prm = cpool.tile([P, 8], f32, name="prm_s")
            nc.sync.dma_start(prm[:], prm_d)
            iota = cpool.tile([P, WIN], mybir.dt.int16, name="iota_s")
            nc.gpsimd.iota(iota[:], pattern=[[1, WIN]], base=0, channel_multiplier=0)
            iotab = cpool.tile([P, WIN], bf, name="iotab_s")
            nc.vector.tensor_copy(iotab[:], iota[:])

            psum_t, cur_blk = None, -1
            for cc in range(C // CCH):
                c0 = cc * CCH
                fs = io.tile([P, CCH], f32, tag="fs")
                fd = io.tile([P, CCH], f32, tag="fd")
                ids = io.tile([P, CCH], bf, tag="ids")
                nc.sync.dma_start(fs[:], fs_d[:, c0:c0 + CCH])
                nc.sync.dma_start(fd[:], fd_d[:, c0:c0 + CCH])
                nc.sync.dma_start(ids[:], ids_d[:, c0:c0 + CCH])

                vi = work.tile([P, 8 * CCH], bf, tag="vi")
                vi3 = vi[:].rearrange("p (v c) -> p v c", v=8)
                t1 = work.tile([P, CCH], f32, tag="t1")
                z = work.tile([P, CCH], f32, tag="z")
                a = work.tile([P, CCH], f32, tag="a")
                fsb = work.tile([P, CCH], bf, tag="fsb")
                nc.vector.tensor_copy(fsb[:], fs[:])
                for h in range(4):
                    nc.vector.tensor_scalar_mul(t1[:], fd[:], prm[:, 4 + h:5 + h])
                    nc.vector.scalar_tensor_tensor(
                        out=z[:], in0=fs[:], scalar=prm[:, h:h + 1], in1=t1[:],
                        op0=mybir.AluOpType.mult, op1=mybir.AluOpType.add)
                    nc.vector.scalar_tensor_tensor(
                        out=a[:], in0=z[:], scalar=0.2, in1=z[:],
                        op0=mybir.AluOpType.mult, op1=mybir.AluOpType.max)
                    nc.scalar.activation(vi3[:, h, :], a[:],
                                         mybir.ActivationFunctionType.Exp)
                    nc.vector.tensor_mul(vi3[:, 4 + h, :], vi3[:, h, :], fsb[:])

                for ch in range(CCH // CHK):
                    t0 = c0 + ch * CHK
                    oh = ohp.tile([P, CHK * WIN], bf, tag="oh")
                    nc.vector.tensor_tensor(
                        out=oh[:].rearrange("p (c w) -> p c w", w=WIN),
                        in0=ids[:, ch * CHK:(ch + 1) * CHK].unsqueeze(-1)
                            .to_broadcast([P, CHK, WIN]),
                        in1=iotab[:].unsqueeze(1).to_broadcast([P, CHK, WIN]),
                        op=mybir.AluOpType.is_equal)
                    for tl in range(CHK):
                        t = t0 + tl
                        w = t // ncw
                        b = w // wpb
                        sup = b // 3
                        if sup != cur_blk:
                            if psum_t is not None:
                                flush(cur_blk, psum_t)
                            psum_t = psum_p.tile([P, BLK], f32, tag="ps")
                            cur_blk = sup
                        wl = w % wpb
                        po = 32 * (b % 3)
                        nc.tensor.matmul(
                            out=psum_t[po:po + 8, wl * WIN:(wl + 1) * WIN],
                            lhsT=vi3[:, :, t - c0],
                            rhs=oh[:, tl * WIN:(tl + 1) * WIN],
                            start=(t % ncw == 0), stop=(t % ncw == ncw - 1))
            flush(cur_blk, psum_t)
    nc.compile()
    return nc


def kernel(features, W, attn_l, attn_r, bias_gat, fc_W, fc_b, src, dst):
    f = np.asarray(features, dtype=np.float32)[:, 0]
    src = np.asarray(src)
    dst = np.asarray(dst)
    N = f.shape[0]
    H, D = np.asarray(attn_l).shape

    nodes_pc = -(-N // NCORES)
    key = ((dst // nodes_pc).astype(np.int64) << 32) | ((dst % nodes_pc) >> 5)
    maxcnt = np.bincount((np.unique(key, return_inverse=True)[1])).max()
    pl = _plan(N, maxcnt)

    W1 = np.asarray(W, np.float64).reshape(H, D)
    cl = (W1 * np.asarray(attn_l, np.float64)).sum(1)
    cr = (W1 * np.asarray(attn_r, np.float64)).sum(1)
    prm = np.zeros((P, 8), dtype=np.float32)
    prm[:, 0:4] = cl.astype(np.float32)
    prm[:, 4:8] = cr.astype(np.float32)

    order = np.argsort(dst, kind="stable")
    ss, dd = src[order], dst[order]
    bounds = np.searchsorted(dd, np.arange(NCORES + 1) * nodes_pc)
    in_maps = []
    for k in range(NCORES):
        a, b = bounds[k], bounds[k + 1]
        arrs = _host_prep_core(f, ss[a:b], dd[a:b], k * nodes_pc, pl)
        in_maps.append({**arrs, "prm": prm})

    nc = _build_program(pl)
    res = bass_utils.run_bass_kernel_spmd(nc, in_maps,
                                          core_ids=list(range(NCORES)),
                                          trace=False)

    ssum = np.zeros(H, dtype=np.float64)
    for k in range(NCORES):
        npc = min(nodes_pc, N - k * nodes_pc)
        raw = res.results[k]["acc"].astype(np.float64)   # [128, nsup*512]
        nsup = raw.shape[1] // BLK
        # p = 32*blk_lo + val (val<8); node = (sup*3 + blk_lo)*512 + j
        r = raw.reshape(4, 32, nsup, BLK)[:3, :8]          # [3, 8, nsup, 512]
        acc = r.transpose(1, 2, 0, 3).reshape(8, -1)[:, :npc]
        denom, num = acc[0:4], acc[4:8]
        s = np.where(denom > 0, num / np.maximum(denom, 1e-300), 0.0)
        ssum += s.sum(axis=1)
    sbar = ssum / N
    rbar = sbar[:, None] * W1 + np.asarray(bias_gat, np.float64).reshape(H, D)
    out = rbar.reshape(1, H * D) @ np.asarray(fc_W, np.float64) \
        + np.asarray(fc_b, np.float64)
    return out[0].astype(np.float32)


# revision 11
# speedup vs baseline: 1.4747x; 1.2426x over previous
"""DGL-GAT subgraph encoder kernel for 8 Trainium2 NeuronCores.

With IN_FEATS=1 the GATConv collapses to per-node scalars:
  feat[n,h,d] = f[n]*W1[h,d];  el[n,h] = f[n]*cl[h];  er[n,h] = f[n]*cr[h]
  w[e,h] = exp(lrelu(f[src]*cl[h] + f[dst]*cr[h]))   (softmax max-shift cancels
  in the num/denom ratio; exponents stay < ~25 so no overflow)
  denom[n,h] = seg_sum_dst(w);  num[n,h] = seg_sum_dst(w * f[src])
  s[n,h] = num/denom;  sbar[h] = mean_n s
  out = (sbar[h]*W1[h,:] + bias_gat) @ fc_W + fc_b     (tiny, done on host)

Sharding: core k owns dst nodes [k*12500, (k+1)*12500) and all edges into
them.  Edges are dst-sorted into window-pure 128-edge columns (32-node
one-hot windows, uniform capacity so all cores share one program).  The
device computes per-edge w and w*fs (DVE/ACT) and the two segment sums via
PE matmuls  V[128e,8]^T x onehot[128e,32] accumulated in [8,512] PSUM
blocks; per-core partial (denom,num) tables return to the host, which does
the 100K-node ratio/mean and the final 256x128 projection.
"""
import numpy as np
import ml_dtypes
import concourse.bass as bass
import concourse.tile as tile
from concourse import bacc, mybir, bass_utils

WIN = 32          # nodes per one-hot window (matmul N)
BLK = 512         # nodes per psum block
P = 128           # edges per column
CHK = 128         # columns per onehot chunk
CCH = 512         # columns per compute/load chunk
NCORES = 8

BF16 = ml_dtypes.bfloat16


def _plan(n_nodes, nwin_max):
    nodes_pc = -(-n_nodes // NCORES)
    ncw = 4
    C = -(-(nwin_max * ncw) // CHK) * CHK
    nblk = ((C - 1) // ncw) // (BLK // WIN) + 1
    return dict(nodes_pc=nodes_pc, nwin=nwin_max, ncw=ncw, C=C, nblk=nblk)


def _pack_windows(deg):
    """Greedy sequential packing: nodes (in order) into windows of <=WIN nodes
    and <=ncw*P edges.  Returns per-node window id and within-window slot."""
    cap = 4 * P
    nodewin = np.empty(len(deg), dtype=np.int64)
    nodeslot = np.empty(len(deg), dtype=np.int64)
    w = nn = ee = 0
    for i, dg in enumerate(deg):
        if nn >= WIN or ee + dg > cap:
            w += 1; nn = 0; ee = 0
        nodewin[i] = w
        nodeslot[i] = nn
        nn += 1; ee += dg
    return nodewin, nodeslot, w + 1


def _host_prep_core(f, src_c, dst_c, lo, pl, nodewin, nodeslot):
    ncw, C = pl["ncw"], pl["C"]
    o = np.argsort(dst_c, kind="stable")
    s_c, d_c = src_c[o], dst_c[o]
    nloc = d_c - lo
    win = nodewin[nloc]
    idl = nodeslot[nloc]
    starts = np.searchsorted(win, np.arange(pl["nwin"]))
    rank = np.arange(len(win)) - starts[win]
    cap = ncw * P
    assert rank.max(initial=0) < cap, "window capacity overflow"
    flat = win * cap + rank

    def scatter(vals, fill, dt):
        a = np.full(C * P, fill, dtype=np.float32)
        a[flat] = vals
        return np.ascontiguousarray(a.reshape(C, P).T).astype(dt)

    return dict(fs=scatter(f[s_c], 0.0, np.float32),
                fd=scatter(f[d_c], 0.0, np.float32),
                ids=scatter(idl.astype(np.float32), -1.0, BF16))


def _build_program(pl):
    C, ncw, nblk = pl["C"], pl["ncw"], pl["nblk"]
    nc = bacc.Bacc("TRN2", target_bir_lowering=False, debug=False,
                   enable_asserts=False, num_devices=NCORES)
    bf = mybir.dt.bfloat16
    f32 = mybir.dt.float32

    fs_d = nc.dram_tensor("fs", [P, C], f32, kind="ExternalInput").ap()
    fd_d = nc.dram_tensor("fd", [P, C], f32, kind="ExternalInput").ap()
    ids_d = nc.dram_tensor("ids", [P, C], bf, kind="ExternalInput").ap()
    prm_d = nc.dram_tensor("prm", [P, 8], f32, kind="ExternalInput").ap()
    nsup = -(-nblk // 3)
    acc_d = nc.dram_tensor("acc", [P, nsup * BLK], f32, kind="ExternalOutput").ap()
    wpb = BLK // WIN

    with tile.TileContext(nc) as tc:
        with tc.tile_pool(name="consts", bufs=1) as cpool, \
             tc.tile_pool(name="io", bufs=3) as io, \
             tc.tile_pool(name="work", bufs=2) as work, \
             tc.tile_pool(name="ohp", bufs=3) as ohp, \
             tc.tile_pool(name="flp", bufs=2) as flp, \
             tc.tile_pool(name="psum", bufs=2, space="PSUM") as psum_p:
            def flush(sup, ps):
                st = flp.tile([P, BLK], f32, tag="fl")
                nc.vector.tensor_copy(st[:], ps[:])
                nc.sync.dma_start(acc_d[:, sup * BLK:(sup + 1) * BLK], st[:])

            prm = cpool.tile([P, 8], f32, name="prm_s")
            nc.sync.dma_start(prm[:], prm_d)
            iota = cpool.tile([P, WIN], mybir.dt.int16, name="iota_s")
            nc.gpsimd.iota(iota[:], pattern=[[1, WIN]], base=0, channel_multiplier=0)
            iotab = cpool.tile([P, WIN], bf, name="iotab_s")
            nc.vector.tensor_copy(iotab[:], iota[:])

            psum_t, cur_blk = None, -1
            chunks = []
            c0x = 0
            while c0x < C:
                chunks.append((c0x, min(CCH, C - c0x)))
                c0x += CCH
            for c0, CL in chunks:
                fst = io.tile([P, CCH], f32, tag="fs")
                fdt = io.tile([P, CCH], f32, tag="fd")
                idst = io.tile([P, CCH], bf, tag="ids")
                fs = fst[:, :CL]; fd = fdt[:, :CL]; ids = idst[:, :CL]
                nc.sync.dma_start(fs, fs_d[:, c0:c0 + CL])
                nc.sync.dma_start(fd, fd_d[:, c0:c0 + CL])
                nc.sync.dma_start(ids, ids_d[:, c0:c0 + CL])

                vi = work.tile([P, 8 * CCH], bf, tag="vi")
                vi3 = vi[:].rearrange("p (v c) -> p v c", v=8)[:, :, :CL]
                t1 = work.tile([P, CCH], f32, tag="t1", name="t1t")[:, :CL]
                z = work.tile([P, CCH], f32, tag="z", name="zt")[:, :CL]
                e1 = work.tile([P, CCH], bf, tag="e1", name="e1t")[:, :CL]
                e2 = work.tile([P, CCH], bf, tag="e2", name="e2t")[:, :CL]
                fsb = work.tile([P, CCH], bf, tag="fsb", name="fsbt")[:, :CL]
                nc.vector.tensor_copy(fsb, fs)
                for h in range(4):
                    nc.vector.tensor_scalar_mul(t1, fd, prm[:, 4 + h:5 + h])
                    nc.vector.scalar_tensor_tensor(
                        out=z, in0=fs, scalar=prm[:, h:h + 1], in1=t1,
                        op0=mybir.AluOpType.mult, op1=mybir.AluOpType.add)
                    nc.vector.scalar_tensor_tensor(
                        out=t1, in0=z, scalar=0.2, in1=z,
                        op0=mybir.AluOpType.mult, op1=mybir.AluOpType.max)
                    nc.scalar.activation(vi3[:, h, :], t1,
                                         mybir.ActivationFunctionType.Exp)
                    nc.vector.tensor_mul(vi3[:, 4 + h, :], vi3[:, h, :], fsb)

                for ch in range(CL // CHK):
                    t0 = c0 + ch * CHK
                    oh = ohp.tile([P, CHK * WIN], bf, tag="oh")
                    nc.vector.tensor_tensor(
                        out=oh[:].rearrange("p (c w) -> p c w", w=WIN),
                        in0=ids[:, ch * CHK:(ch + 1) * CHK].unsqueeze(-1)
                            .to_broadcast([P, CHK, WIN]),
                        in1=iotab[:].unsqueeze(1).to_broadcast([P, CHK, WIN]),
                        op=mybir.AluOpType.is_equal)
                    for tl in range(CHK):
                        t = t0 + tl
                        w = t // ncw
                        b = w // wpb
                        sup = b // 3
                        if sup != cur_blk:
                            if psum_t is not None:
                                flush(cur_blk, psum_t)
                            psum_t = psum_p.tile([P, BLK], f32, tag="ps")
                            cur_blk = sup
                        wl = w % wpb
                        po = 32 * (b % 3)
                        nc.tensor.matmul(
                            out=psum_t[po:po + 8, wl * WIN:(wl + 1) * WIN],
                            lhsT=vi3[:, :, t - c0],
                            rhs=oh[:, tl * WIN:(tl + 1) * WIN],
                            start=(t % ncw == 0), stop=(t % ncw == ncw - 1))
            flush(cur_blk, psum_t)
    nc.compile()
    return nc


def kernel(features, W, attn_l, attn_r, bias_gat, fc_W, fc_b, src, dst):
    f = np.asarray(features, dtype=np.float32)[:, 0]
    src = np.asarray(src)
    dst = np.asarray(dst)
    N = f.shape[0]
    H, D = np.asarray(attn_l).shape

    nodes_pc = -(-N // NCORES)
    packs = []
    for k in range(NCORES):
        lo = k * nodes_pc
        npc = min(nodes_pc, N - lo)
        deg = np.bincount(dst[(dst >= lo) & (dst < lo + npc)] - lo, minlength=npc)
        packs.append(_pack_windows(deg))
    pl = _plan(N, max(pk[2] for pk in packs))

    W1 = np.asarray(W, np.float64).reshape(H, D)
    cl = (W1 * np.asarray(attn_l, np.float64)).sum(1)
    cr = (W1 * np.asarray(attn_r, np.float64)).sum(1)
    prm = np.zeros((P, 8), dtype=np.float32)
    prm[:, 0:4] = cl.astype(np.float32)
    prm[:, 4:8] = cr.astype(np.float32)

    order = np.argsort(dst, kind="stable")
    ss, dd = src[order], dst[order]
    bounds = np.searchsorted(dd, np.arange(NCORES + 1) * nodes_pc)
    in_maps = []
    for k in range(NCORES):
        a, b = bounds[k], bounds[k + 1]
        arrs = _host_prep_core(f, ss[a:b], dd[a:b], k * nodes_pc, pl,
                               packs[k][0], packs[k][1])
        in_maps.append({**arrs, "prm": prm})

    nc = _build_program(pl)
    res = bass_utils.run_bass_kernel_spmd(nc, in_maps,
                                          core_ids=list(range(NCORES)),
                                          trace=False)

    ssum = np.zeros(H, dtype=np.float64)
    for k in range(NCORES):
        raw = res.results[k]["acc"].astype(np.float64)   # [128, nsup*512]
        nsup = raw.shape[1] // BLK
        # p = 32*blk_lo + val (val<8); slot = (sup*3 + blk_lo)*512 + j
        r = raw.reshape(4, 32, nsup, BLK)[:3, :8]          # [3, 8, nsup, 512]
        acc = r.transpose(1, 2, 0, 3).reshape(8, -1)[:, :pl["nblk"] * BLK]
        denom, num = acc[0:4], acc[4:8]
        s = np.where(denom > 0, num / np.maximum(denom, 1e-300), 0.0)
        ssum += s.sum(axis=1)
    sbar = ssum / N
    rbar = sbar[:, None] * W1 + np.asarray(bias_gat, np.float64).reshape(H, D)
    out = rbar.reshape(1, H * D) @ np.asarray(fc_W, np.float64) \
        + np.asarray(fc_b, np.float64)
    return out[0].astype(np.float32)


# revision 12
# speedup vs baseline: 1.6957x; 1.1498x over previous
"""DGL-GAT subgraph encoder kernel for 8 Trainium2 NeuronCores.

With IN_FEATS=1 the GATConv collapses to per-node scalars:
  feat[n,h,d] = f[n]*W1[h,d];  el[n,h] = f[n]*cl[h];  er[n,h] = f[n]*cr[h]
  w[e,h] = exp(lrelu(f[src]*cl[h] + f[dst]*cr[h]))   (softmax max-shift cancels
  in the num/denom ratio; exponents stay < ~25 so no overflow)
  denom[n,h] = seg_sum_dst(w);  num[n,h] = seg_sum_dst(w * f[src])
  s[n,h] = num/denom;  sbar[h] = mean_n s
  out = (sbar[h]*W1[h,:] + bias_gat) @ fc_W + fc_b     (tiny, done on host)

Sharding: core k owns dst nodes [k*12500, (k+1)*12500) and all edges into
them.  Edges are dst-sorted into window-pure 128-edge columns (32-node
one-hot windows, uniform capacity so all cores share one program).  The
device computes per-edge w and w*fs (DVE/ACT) and the two segment sums via
PE matmuls  V[128e,8]^T x onehot[128e,32] accumulated in [8,512] PSUM
blocks; per-core partial (denom,num) tables return to the host, which does
the 100K-node ratio/mean and the final 256x128 projection.
"""
import numpy as np
import ml_dtypes
import concourse.bass as bass
import concourse.tile as tile
from concourse import bacc, mybir, bass_utils

WIN = 8           # nodes per one-hot window (matmul N)
BLK = 512         # nodes per psum block
P = 128           # edges per column
CHK = 128         # columns per onehot chunk
CCH = 512         # columns per compute/load chunk
NCORES = 8

BF16 = ml_dtypes.bfloat16


def _plan(n_nodes, nwin_max):
    nodes_pc = -(-n_nodes // NCORES)
    ncw = 1
    C = -(-(nwin_max * ncw) // CHK) * CHK
    nblk = ((C - 1) // ncw) // (BLK // WIN) + 1
    return dict(nodes_pc=nodes_pc, nwin=nwin_max, ncw=ncw, C=C, nblk=nblk)


def _pack_windows(deg):
    """Greedy sequential packing: nodes (in order) into windows of <=WIN nodes
    and <=ncw*P edges.  Returns per-node window id and within-window slot."""
    cap = 1 * P
    nodewin = np.empty(len(deg), dtype=np.int64)
    nodeslot = np.empty(len(deg), dtype=np.int64)
    w = nn = ee = 0
    for i, dg in enumerate(deg):
        if nn >= WIN or ee + dg > cap:
            w += 1; nn = 0; ee = 0
        nodewin[i] = w
        nodeslot[i] = nn
        nn += 1; ee += dg
    return nodewin, nodeslot, w + 1


def _host_prep_core(f, src_c, dst_c, lo, pl, nodewin, nodeslot):
    ncw, C = pl["ncw"], pl["C"]
    o = np.argsort(dst_c, kind="stable")
    s_c, d_c = src_c[o], dst_c[o]
    nloc = d_c - lo
    win = nodewin[nloc]
    idl = nodeslot[nloc]
    starts = np.searchsorted(win, np.arange(pl["nwin"]))
    rank = np.arange(len(win)) - starts[win]
    cap = ncw * P
    assert rank.max(initial=0) < cap, "window capacity overflow"
    flat = win * cap + rank

    def scatter(vals, fill, dt):
        a = np.full(C * P, fill, dtype=np.float32)
        a[flat] = vals
        return np.ascontiguousarray(a.reshape(C, P).T).astype(dt)

    return dict(fs=scatter(f[s_c], 0.0, np.float32),
                fd=scatter(f[d_c], 0.0, np.float32),
                ids=scatter(idl.astype(np.float32), -1.0, BF16))


def _build_program(pl):
    C, ncw, nblk = pl["C"], pl["ncw"], pl["nblk"]
    nc = bacc.Bacc("TRN2", target_bir_lowering=False, debug=False,
                   enable_asserts=False, num_devices=NCORES)
    bf = mybir.dt.bfloat16
    f32 = mybir.dt.float32

    fs_d = nc.dram_tensor("fs", [P, C], f32, kind="ExternalInput").ap()
    fd_d = nc.dram_tensor("fd", [P, C], f32, kind="ExternalInput").ap()
    ids_d = nc.dram_tensor("ids", [P, C], bf, kind="ExternalInput").ap()
    prm_d = nc.dram_tensor("prm", [P, 8], f32, kind="ExternalInput").ap()
    nsup = -(-nblk // 3)
    acc_d = nc.dram_tensor("acc", [P, nsup * BLK], f32, kind="ExternalOutput").ap()
    wpb = BLK // WIN

    with tile.TileContext(nc) as tc:
        with tc.tile_pool(name="consts", bufs=1) as cpool, \
             tc.tile_pool(name="io", bufs=3) as io, \
             tc.tile_pool(name="work", bufs=2) as work, \
             tc.tile_pool(name="ohp", bufs=3) as ohp, \
             tc.tile_pool(name="flp", bufs=2) as flp, \
             tc.tile_pool(name="psum", bufs=2, space="PSUM") as psum_p:
            def flush(sup, ps):
                st = flp.tile([P, BLK], f32, tag="fl")
                nc.vector.tensor_copy(st[:], ps[:])
                nc.sync.dma_start(acc_d[:, sup * BLK:(sup + 1) * BLK], st[:])

            prm = cpool.tile([P, 8], f32, name="prm_s")
            nc.sync.dma_start(prm[:], prm_d)
            iota = cpool.tile([P, WIN], mybir.dt.int16, name="iota_s")
            nc.gpsimd.iota(iota[:], pattern=[[1, WIN]], base=0, channel_multiplier=0)
            iotab = cpool.tile([P, WIN], bf, name="iotab_s")
            nc.vector.tensor_copy(iotab[:], iota[:])

            psum_t, cur_blk = None, -1
            chunks = []
            c0x = 0
            while c0x < C:
                chunks.append((c0x, min(CCH, C - c0x)))
                c0x += CCH
            for c0, CL in chunks:
                fst = io.tile([P, CCH], f32, tag="fs")
                fdt = io.tile([P, CCH], f32, tag="fd")
                idst = io.tile([P, CCH], bf, tag="ids")
                fs = fst[:, :CL]; fd = fdt[:, :CL]; ids = idst[:, :CL]
                nc.sync.dma_start(fs, fs_d[:, c0:c0 + CL])
                nc.sync.dma_start(fd, fd_d[:, c0:c0 + CL])
                nc.sync.dma_start(ids, ids_d[:, c0:c0 + CL])

                vi = work.tile([P, 8 * CCH], bf, tag="vi")
                vi3 = vi[:].rearrange("p (v c) -> p v c", v=8)[:, :, :CL]
                t1 = work.tile([P, CCH], f32, tag="t1", name="t1t")[:, :CL]
                z = work.tile([P, CCH], f32, tag="z", name="zt")[:, :CL]
                e1 = work.tile([P, CCH], bf, tag="e1", name="e1t")[:, :CL]
                e2 = work.tile([P, CCH], bf, tag="e2", name="e2t")[:, :CL]
                fsb = work.tile([P, CCH], bf, tag="fsb", name="fsbt")[:, :CL]
                nc.vector.tensor_copy(fsb, fs)
                for h in range(4):
                    nc.vector.tensor_scalar_mul(t1, fd, prm[:, 4 + h:5 + h])
                    nc.vector.scalar_tensor_tensor(
                        out=z, in0=fs, scalar=prm[:, h:h + 1], in1=t1,
                        op0=mybir.AluOpType.mult, op1=mybir.AluOpType.add)
                    nc.scalar.activation(e1, z, mybir.ActivationFunctionType.Exp)
                    nc.scalar.activation(e2, z, mybir.ActivationFunctionType.Exp,
                                         scale=0.2)
                    nc.vector.tensor_tensor(out=vi3[:, h, :], in0=e1, in1=e2,
                                            op=mybir.AluOpType.max)
                    nc.vector.tensor_mul(vi3[:, 4 + h, :], vi3[:, h, :], fsb)

                for ch in range(CL // CHK):
                    t0 = c0 + ch * CHK
                    oh = ohp.tile([P, CHK * WIN], bf, tag="oh")
                    nc.vector.tensor_tensor(
                        out=oh[:].rearrange("p (c w) -> p c w", w=WIN),
                        in0=ids[:, ch * CHK:(ch + 1) * CHK].unsqueeze(-1)
                            .to_broadcast([P, CHK, WIN]),
                        in1=iotab[:].unsqueeze(1).to_broadcast([P, CHK, WIN]),
                        op=mybir.AluOpType.is_equal)
                    for tl in range(CHK):
                        t = t0 + tl
                        w = t // ncw
                        b = w // wpb
                        sup = b // 3
                        if sup != cur_blk:
                            if psum_t is not None:
                                flush(cur_blk, psum_t)
                            psum_t = psum_p.tile([P, BLK], f32, tag="ps")
                            cur_blk = sup
                        wl = w % wpb
                        po = 32 * (b % 3)
                        nc.tensor.matmul(
                            out=psum_t[po:po + 8, wl * WIN:(wl + 1) * WIN],
                            lhsT=vi3[:, :, t - c0],
                            rhs=oh[:, tl * WIN:(tl + 1) * WIN],
                            start=(t % ncw == 0), stop=(t % ncw == ncw - 1))
            flush(cur_blk, psum_t)
    nc.compile()
    return nc


def kernel(features, W, attn_l, attn_r, bias_gat, fc_W, fc_b, src, dst):
    f = np.asarray(features, dtype=np.float32)[:, 0]
    src = np.asarray(src)
    dst = np.asarray(dst)
    N = f.shape[0]
    H, D = np.asarray(attn_l).shape

    nodes_pc = -(-N // NCORES)
    packs = []
    for k in range(NCORES):
        lo = k * nodes_pc
        npc = min(nodes_pc, N - lo)
        deg = np.bincount(dst[(dst >= lo) & (dst < lo + npc)] - lo, minlength=npc)
        packs.append(_pack_windows(deg))
    pl = _plan(N, max(pk[2] for pk in packs))

    W1 = np.asarray(W, np.float64).reshape(H, D)
    cl = (W1 * np.asarray(attn_l, np.float64)).sum(1)
    cr = (W1 * np.asarray(attn_r, np.float64)).sum(1)
    prm = np.zeros((P, 8), dtype=np.float32)
    prm[:, 0:4] = cl.astype(np.float32)
    prm[:, 4:8] = cr.astype(np.float32)

    order = np.argsort(dst, kind="stable")
    ss, dd = src[order], dst[order]
    bounds = np.searchsorted(dd, np.arange(NCORES + 1) * nodes_pc)
    in_maps = []
    for k in range(NCORES):
        a, b = bounds[k], bounds[k + 1]
        arrs = _host_prep_core(f, ss[a:b], dd[a:b], k * nodes_pc, pl,
                               packs[k][0], packs[k][1])
        in_maps.append({**arrs, "prm": prm})

    nc = _build_program(pl)
    res = bass_utils.run_bass_kernel_spmd(nc, in_maps,
                                          core_ids=list(range(NCORES)),
                                          trace=False)

    ssum = np.zeros(H, dtype=np.float64)
    for k in range(NCORES):
        raw = res.results[k]["acc"].astype(np.float64)   # [128, nsup*512]
        nsup = raw.shape[1] // BLK
        # p = 32*blk_lo + val (val<8); slot = (sup*3 + blk_lo)*512 + j
        r = raw.reshape(4, 32, nsup, BLK)[:3, :8]          # [3, 8, nsup, 512]
        acc = r.transpose(1, 2, 0, 3).reshape(8, -1)[:, :pl["nblk"] * BLK]
        denom, num = acc[0:4], acc[4:8]
        s = np.where(denom > 0, num / np.maximum(denom, 1e-300), 0.0)
        ssum += s.sum(axis=1)
    sbar = ssum / N
    rbar = sbar[:, None] * W1 + np.asarray(bias_gat, np.float64).reshape(H, D)
    out = rbar.reshape(1, H * D) @ np.asarray(fc_W, np.float64) \
        + np.asarray(fc_b, np.float64)
    return out[0].astype(np.float32)
